# revision 24
# baseline (speedup 1.0000x reference)
"""Trainium2 Bass kernel: single transformer layer (attn + gated MLP, LoRA on all
projections), B=4 S=1024 D=2048 H=16 HD=128 FF=8192, fp32 in/out.

Sharding (8 cores, no collectives): core c -> batch b=c//2, q-row chunks
{0,3} (c%2==0) or {1,2} (c%2==1) of 256 rows each (causally balanced).
Each core computes k/v for its full batch (duplicated within the pair), its
own 512 q-rows through attention + o-proj, and the row-sharded MLP.

v3: LoRA folded into the dense weights host-side (x@W + (x@A)@B = x@(W+A@B));
weights streamed as bf16 stationaries (fp32r moving activations keep full
matmul speed); the middle of the kernel stays in transposed [D-part, row]
space (residual reconstructed as n1T/rstd1, rmsnorm2 via on-the-fly sum of
squares + gpsimd partition reduce, rstd2 applied after the gate/up matmuls);
transposes are regular matmuls against a bf16 identity (or rstd-scaled
diagonal, fusing the rmsnorm1 scale into the transpose); stage0 is
interleaved with the first head's k/v work. Norm weights and the 1/sqrt(HD)
score scale are folded into the projection weights host-side.
"""
import numpy as np
from contextlib import ExitStack

import concourse.bass as bass
import concourse.tile as tile
import concourse.mybir as mybir
from concourse import bacc
import concourse.bass_isa as bass_isa
from concourse.bass_utils import run_bass_kernel_spmd

F32 = mybir.dt.float32
F32R = mybir.dt.float32r
BF16 = mybir.dt.bfloat16
AL = mybir.AluOpType
AF = mybir.ActivationFunctionType

B, S, D = 4, 1024, 2048
H, HD = 16, 128
FF, LR = 8192, 16
EPS = 1e-5
CH = 256              # q-chunk rows
DB = D // 128         # 16
SB = S // 128         # 8
FFB = FF // 128       # 64
FFQ = 16              # ff blocks per quarter
CHUNKS = [[0, 3], [1, 2]]  # global q-chunk ids per half

_CACHE = {}


def _emit(nc, tc, ap, half):
    chunks = CHUNKS[half]
    own_rb = [rb for qc in chunks for rb in (2 * qc, 2 * qc + 1)]

    def _own(ap2d):
        # [P, S] -> [P, 2, CH] strided view selecting this half's chunks
        a = ap2d.rearrange("p (a c) -> p a c", c=CH)
        return a[:, ::3, :] if chunks == [0, 3] else a[:, 1:3, :]

    ctx = ExitStack()
    with ctx:
        persist = ctx.enter_context(tc.tile_pool(name="persist", bufs=1))
        small = ctx.enter_context(tc.tile_pool(name="small", bufs=3))

        identb = persist.tile([128, 128], BF16)
        nc.sync.dma_start(out=identb, in_=ap["identb"])
        ident_r = persist.tile([128, 128], F32R)
        nc.sync.dma_start(out=ident_r, in_=ap["identr"])
        ones_r = persist.tile([1, 128], F32R)
        nc.sync.dma_start(out=ones_r, in_=ap["onesr"])
        eps_t = persist.tile([128, 1], F32)
        nc.vector.memset(eps_t, EPS)
        rotT = persist.tile([128, 128], F32R)
        cosT = persist.tile([HD, S], F32)
        sinT = persist.tile([HD, S], F32)
        maskT = persist.tile([128, 2, 2, CH], F32)
        rinv_nat = persist.tile([128, 4], F32R)
        xsq = persist.tile([128, D], BF16)         # Square scratch (unused values)
        rinv_bc = persist.tile([128, 512], F32)    # 1/rstd1 broadcast [d, own_r]

        p_x1 = ctx.enter_context(tc.tile_pool(name="p_x1", bufs=1))
        mlppre = ctx.enter_context(tc.tile_pool(name="mlppre", bufs=1))

        with tc.tile_pool(name="p_oT", bufs=1) as p_oT:
            oT = p_oT.tile([128, H, 512], BF16)  # [hd, head, own_q] 2MB
            attw_cm = tc.tile_pool(name="attw", bufs=6)
            attw = attw_cm.__enter__()

            with tc.tile_pool(name="p_h1", bufs=1) as p_h1:
                n1T = p_h1.tile([128, DB, S], BF16)   # 4MB, x*rstd1 transposed

                with tc.tile_pool(name="att", bufs=1) as att, \
                     tc.tile_pool(name="attv", bufs=3) as attv, \
                     tc.tile_pool(name="attk", bufs=3) as attk, \
                     tc.tile_pool(name="attp2", bufs=2) as attp2, \
                     tc.tile_pool(name="attkp", bufs=3, space="PSUM") as attkp, \
                     tc.tile_pool(name="atttp", bufs=2, space="PSUM") as atttp, \
                     tc.tile_pool(name="attop", bufs=1, space="PSUM") as attop, \
                     tc.tile_pool(name="attps", bufs=2, space="PSUM") as attps:

                    def dma_w(name, h, eng=None):
                        t = attw.tile([128, DB, 128], BF16, tag="w")
                        (eng or nc.sync).dma_start(out=t, in_=ap[name][h]
                                                   .rearrange("p (db m) -> p db m", m=128))
                        return t

                    def emit_k_nc2(nc2, wk_t, kT_h):
                        ssl = slice(nc2 * 512, (nc2 + 1) * 512)
                        kp = attkp.tile([128, 512], F32, tag="kp")
                        for db in range(DB):
                            nc.tensor.matmul(kp[:], wk_t[:, db, :], n1T[:, db, ssl],
                                             start=(db == 0), stop=False)
                        tsin = att.tile([128, 512], F32R, tag="tsin")
                        nc.vector.tensor_tensor(out=tsin[:], in0=kp[:],
                                                in1=sinT[:, ssl], op=AL.mult)
                        nc.vector.tensor_tensor(out=kp[:], in0=kp[:],
                                                in1=cosT[:, ssl], op=AL.mult)
                        nc.tensor.matmul(kp[:], rotT[:], tsin[:],
                                         start=False, stop=True, skip_group_check=True)
                        nc.scalar.copy(kT_h[:, ssl], kp[:])

                    def emit_v_nc2(nc2, wv_t, v_nat):
                        ssl = slice(nc2 * 512, (nc2 + 1) * 512)
                        vp = attkp.tile([128, 512], F32, tag="kp")
                        for db in range(DB):
                            nc.tensor.matmul(vp[:], wv_t[:, db, :], n1T[:, db, ssl],
                                             start=(db == 0), stop=(db == DB - 1))
                        vT_sb = att.tile([128, 512], BF16, tag="vT_sb")
                        nc.scalar.copy(vT_sb[:], vp[:])
                        tp = atttp.tile([128, 4, 128], BF16, tag="tp")
                        for j in range(4):
                            nc.tensor.matmul(tp[:, j, :], vT_sb[:, j * 128:(j + 1) * 128],
                                             identb[:], is_transpose=True,
                                             start=True, stop=True,
                                             skip_group_check=True)
                        nc.vector.tensor_copy(v_nat[:, nc2 * 4:(nc2 + 1) * 4, :],
                                              tp[:])

                    x_pre = {}

                    # ---- stage 0: fused rmsnorm1 + scale-transpose -> n1T ----
                    def emit_stage0_rb(st0, rb):
                        x_t = x_pre.pop(rb, None)
                        if x_t is None:
                            x_t = st0.tile([128, D], F32, tag="x_t")
                            nc.sync.dma_start(
                                out=x_t, in_=ap["xb"][rb * 128:(rb + 1) * 128, :])
                        acc = st0.tile([128, 1], F32, tag="acc")
                        nc.scalar.activation(out=xsq[:], in_=x_t[:], func=AF.Square,
                                             accum_out=acc[:])
                        sd = st0.tile([128, 1], F32, tag="sd")
                        nc.scalar.activation(out=sd[:], in_=acc[:], func=AF.Sqrt,
                                             bias=eps_t[:], scale=1.0 / D)
                        rstd = st0.tile([128, 1], F32, tag="rstd")
                        nc.vector.reciprocal(out=rstd[:], in_=sd[:])
                        h1b = st0.tile([128, D], BF16, tag="h1b")
                        nc.vector.scalar_tensor_tensor(
                            out=h1b[:], in0=x_t[:], scalar=rstd[:], in1=x_t[:],
                            op0=AL.mult, op1=AL.bypass)
                        if rb in own_rb:
                            j = own_rb.index(rb)
                            rinv_f = st0.tile([128, 1], F32, tag="rinvf")
                            nc.vector.reciprocal(out=rinv_f[:], in_=rstd[:])
                            nc.scalar.copy(rinv_nat[:, j:j + 1], rinv_f[:])
                        for g in range(4):
                            tp = atttp.tile([128, 4, 128], BF16, tag="tp")
                            for j in range(4):
                                db = g * 4 + j
                                nc.tensor.matmul(tp[:, j, :],
                                                 h1b[:, db * 128:(db + 1) * 128],
                                                 identb[:], is_transpose=True,
                                                 start=True, stop=True,
                                                 skip_group_check=True)
                            nc.vector.tensor_copy(
                                n1T[:, g * 4:(g + 1) * 4, rb * 128:(rb + 1) * 128],
                                tp[:])

                    kT0 = attk.tile([128, S], F32R, tag="kT")
                    vnat0 = attv.tile([128, SB, 128], BF16, tag="v_nat")
                    with tc.tile_pool(name="st0", bufs=2) as st0:
                        # first x block ahead of the head-0 weights on the queue
                        xt0 = st0.tile([128, D], F32, tag="x_t")
                        nc.sync.dma_start(out=xt0, in_=ap["xb"][0:128, :])
                        x_pre[0] = xt0
                        wk0 = dma_w("wk", 0)
                        wv0 = dma_w("wv", 0)
                        for rb in range(4):
                            emit_stage0_rb(st0, rb)
                        nc.sync.dma_start(out=cosT, in_=ap["cosT"])
                        nc.sync.dma_start(out=sinT, in_=ap["sinT"])
                        nc.sync.dma_start(out=rotT, in_=ap["rotT"])
                        # head-0/1 k/v over the first 512 rows fill the PE
                        # while stage 0 finishes the remaining row blocks
                        emit_k_nc2(0, wk0, kT0)
                        emit_v_nc2(0, wv0, vnat0)
                        wk1 = dma_w("wk", 1)
                        wv1 = dma_w("wv", 1)
                        kT1 = attk.tile([128, S], F32R, tag="kT")
                        vnat1 = attv.tile([128, SB, 128], BF16, tag="v_nat")
                        emit_stage0_rb(st0, 4)
                        emit_k_nc2(0, wk1, kT1)
                        emit_stage0_rb(st0, 5)
                        emit_v_nc2(0, wv1, vnat1)
                        wk2 = dma_w("wk", 2)
                        wv2 = dma_w("wv", 2)
                        kT2 = attk.tile([128, S], F32R, tag="kT")
                        vnat2 = attv.tile([128, SB, 128], BF16, tag="v_nat")
                        emit_stage0_rb(st0, 6)
                        emit_k_nc2(0, wk2, kT2)
                        emit_stage0_rb(st0, 7)
                        emit_v_nc2(0, wv2, vnat2)
                        nc.sync.dma_start(out=maskT,
                                          in_=ap["maskT"].rearrange("c k p q -> p c k q"))

                    # broadcast 1/rstd1 of own rows into [d-part, own_r] layout
                    with tc.tile_pool(name="rbc", bufs=1) as rbc:
                        for j in range(4):
                            rT_full = attps.tile([128, 2, CH], F32R, tag="sp")
                            rT_ps = rT_full[0:1, 0, 0:128]
                            nc.tensor.matmul(rT_ps, rinv_nat[:, j:j + 1],
                                             ident_r[:], is_transpose=True,
                                             start=True, stop=True,
                                             skip_group_check=True)
                            rT_s = rbc.tile([1, 128], F32R, tag="rTs")
                            nc.scalar.copy(rT_s[:], rT_ps)
                            bc_full = attop.tile([128, 2, CH], F32, tag="op")
                            bc_ps = bc_full[:, 0, 0:128]
                            nc.tensor.matmul(bc_ps, ones_r[:], rT_s[:],
                                             start=True, stop=True,
                                             skip_group_check=True)
                            nc.scalar.copy(rinv_bc[:, j * 128:(j + 1) * 128], bc_ps)

                    # ---- per-head attention ----
                    wo0 = None
                    for h in range(H):
                        if h == H - 1:
                            wo0 = attw.tile([128, H, 128], BF16, tag="w")
                            nc.sync.dma_start(out=wo0,
                                              in_=ap["wo"][0]
                                              .rearrange("p (hb m) -> p hb m", m=128))
                        if h == 0:
                            wq_t = dma_w("wq", 0)
                            kT_h, v_nat = kT0, vnat0
                            emit_k_nc2(1, wk0, kT_h)
                            emit_v_nc2(1, wv0, v_nat)
                        elif h == 1:
                            wq_t = dma_w("wq", 1)
                            kT_h, v_nat = kT1, vnat1
                            emit_k_nc2(1, wk1, kT_h)
                            emit_v_nc2(1, wv1, v_nat)
                        elif h == 2:
                            wq_t = dma_w("wq", 2)
                            kT_h, v_nat = kT2, vnat2
                            emit_k_nc2(1, wk2, kT_h)
                            emit_v_nc2(1, wv2, v_nat)
                        else:
                            wk_t = dma_w("wk", h)
                            wv_t = dma_w("wv", h)
                            wq_t = dma_w("wq", h)
                            kT_h = attk.tile([128, S], F32R, tag="kT")
                            v_nat = attv.tile([128, SB, 128], BF16, tag="v_nat")
                            emit_k_nc2(0, wk_t, kT_h)
                            emit_k_nc2(1, wk_t, kT_h)
                            emit_v_nc2(0, wv_t, v_nat)
                            emit_v_nc2(1, wv_t, v_nat)

                        # q'T own rows (one N=512 group) with fused RoPE
                        qT_h = att.tile([128, 2 * CH], F32R, tag="qT")
                        qp = attkp.tile([128, 512], F32, tag="kp")
                        qpv = qp[:].rearrange("p (a c) -> p a c", c=CH)
                        for db in range(DB):
                            nc.tensor.matmul(qpv, wq_t[:, db, :],
                                             _own(n1T[:, db, :]),
                                             start=(db == 0), stop=False)
                        tsin = att.tile([128, 512], F32R, tag="tsin")
                        tsv = tsin[:].rearrange("p (a c) -> p a c", c=CH)
                        nc.vector.tensor_tensor(out=tsv, in0=qpv,
                                                in1=_own(sinT[:]), op=AL.mult)
                        nc.vector.tensor_tensor(out=qpv, in0=qpv,
                                                in1=_own(cosT[:]), op=AL.mult)
                        nc.tensor.matmul(qp[:], rotT[:], tsin[:],
                                         start=False, stop=True, skip_group_check=True)
                        nc.scalar.copy(qT_h[:], qp[:])

                        # scores / softmax / AV per chunk
                        op2 = attop.tile([128, 2, CH], F32, tag="op")
                        for ci, qc in enumerate(chunks):
                            nkb = 2 * qc + 2
                            osl = slice(ci * CH, (ci + 1) * CH)
                            p_sb = attp2.tile([128, SB, CH], BF16, tag="p_sb")
                            for kb in range(nkb):
                                if kb % 2 == 0:
                                    sp2 = attps.tile([128, 2, CH], F32, tag="sp")
                                sp = sp2[:, kb % 2, :]
                                nc.tensor.matmul(sp, kT_h[:, kb * 128:(kb + 1) * 128],
                                                 qT_h[:, osl], start=True, stop=True,
                                                 skip_group_check=True)
                                j = kb - 2 * qc
                                if j >= 0:
                                    nc.vector.tensor_tensor(out=sp, in0=sp,
                                                            in1=maskT[:, ci, j, :], op=AL.add)
                                nc.scalar.activation(out=p_sb[:, kb, :], in_=sp, func=AF.Exp)
                            den = att.tile([128, CH], F32, tag="den")
                            nc.vector.tensor_tensor(out=den[:], in0=p_sb[:, 0, :],
                                                    in1=p_sb[:, 1, :], op=AL.add)
                            for kb in range(2, nkb):
                                nc.vector.tensor_tensor(out=den[:], in0=den[:],
                                                        in1=p_sb[:, kb, :],
                                                        op=AL.add)
                            dall = small.tile([128, CH], F32, tag="dall")
                            nc.gpsimd.partition_all_reduce(dall[:], den[:], 128,
                                                           bass_isa.ReduceOp.add)
                            dbc = small.tile([128, CH], F32, tag="dbc_sb")
                            nc.vector.reciprocal(out=dbc[:], in_=dall[:])
                            op_ = op2[:, ci, :]
                            for kb in range(nkb):
                                nc.tensor.matmul(op_, v_nat[:, kb, :], p_sb[:, kb, :],
                                                 start=(kb == 0), stop=(kb == nkb - 1),
                                                 skip_group_check=True)
                            nc.vector.tensor_tensor(out=oT[:, h, osl], in0=op_,
                                                    in1=dbc[:], op=AL.mult)

                # ---- stage 3: o-proj + residual -> x1T (stays transposed);
                #      accumulate sum-of-squares for rmsnorm2 on the fly ----
                x1T = p_x1.tile([128, DB, 512], BF16)  # post-attn residual, transposed
                ssq = p_x1.tile([128, 512], F32)       # per-partition sum of x1^2
                wg_pre = mlppre.tile([128, DB, 128], BF16)
                wu_pre = mlppre.tile([128, DB, 128], BF16)
                nc.scalar.dma_start(out=wg_pre, in_=ap["wg"][0]
                                    .rearrange("p (db m) -> p db m", m=128))
                nc.scalar.dma_start(out=wu_pre, in_=ap["wu"][0]
                                    .rearrange("p (db m) -> p db m", m=128))
                with tc.tile_pool(name="st3", bufs=2) as st3, \
                     tc.tile_pool(name="st3ps", bufs=3, space="PSUM") as st3ps:
                    for db in range(DB):
                        if db == 0:
                            wo_t = wo0
                        else:
                            wo_t = attw.tile([128, H, 128], BF16, tag="w")
                            nc.sync.dma_start(out=wo_t,
                                              in_=ap["wo"][db]
                                              .rearrange("p (hb m) -> p hb m", m=128))
                        xp = st3ps.tile([128, 512], F32, tag="xp")
                        for hb in range(H):
                            nc.tensor.matmul(xp[:], wo_t[:, hb, :], oT[:, hb, :],
                                             start=(hb == 0), stop=(hb == H - 1))
                        xres = st3.tile([128, 2, CH], F32, tag="xres")
                        nc.vector.tensor_tensor(out=xres[:],
                                                in0=_own(n1T[:, db, :]),
                                                in1=rinv_bc[:].rearrange("p (a c) -> p a c", c=CH),
                                                op=AL.mult)
                        nc.vector.tensor_tensor(out=x1T[:, db, :], in0=xp[:],
                                                in1=xres[:].rearrange("p a c -> p (a c)"),
                                                op=AL.add)
                        sq = st3.tile([128, 512], F32, tag="sq")
                        nc.vector.tensor_tensor(out=sq[:], in0=x1T[:, db, :],
                                                in1=x1T[:, db, :], op=AL.mult)
                        if db == 0:
                            nc.scalar.copy(ssq[:], sq[:])
                        else:
                            nc.vector.tensor_tensor(out=ssq[:], in0=ssq[:],
                                                    in1=sq[:], op=AL.add)
            attw_cm.__exit__(None, None, None)

        # ---- stage 4: MLP on x1T; rstd2 applied after the gate/up matmuls so
        #      they can start before the partition reduce finishes ----
        with tc.tile_pool(name="rs2", bufs=1) as rs2:
            ssq_bc = rs2.tile([128, 512], F32)
            nc.gpsimd.partition_all_reduce(ssq_bc[:], ssq[:], 128,
                                           bass_isa.ReduceOp.add)
            sd2 = rs2.tile([128, 512], F32)
            nc.scalar.activation(out=sd2[:], in_=ssq_bc[:], func=AF.Sqrt,
                                 bias=eps_t[:], scale=1.0 / D)
            rstd2_bc = rs2.tile([128, 512], F32)
            nc.vector.reciprocal(out=rstd2_bc[:], in_=sd2[:])

            had = rs2.tile([128, FFQ, 512], BF16)     # 2MB (per quarter)
            x2T = rs2.tile([128, DB, 512], F32R)      # 4MB

            with tc.tile_pool(name="mlpw", bufs=4) as mlpw, \
                 tc.tile_pool(name="mlps", bufs=2) as mlps, \
                 tc.tile_pool(name="st5", bufs=3) as st5, \
                 tc.tile_pool(name="mlpps", bufs=3, space="PSUM") as mlpps, \
                 tc.tile_pool(name="st5ps", bufs=2, space="PSUM") as st5ps:
                for qt in range(4):
                    for fi in range(FFQ):
                        fb = qt * FFQ + fi
                        if fb == 0:
                            wg_t, wu_t = wg_pre, wu_pre
                        else:
                            wg_t = mlpw.tile([128, DB, 128], BF16, tag="w")
                            nc.sync.dma_start(out=wg_t, in_=ap["wg"][fb]
                                              .rearrange("p (db m) -> p db m", m=128))
                            wu_t = mlpw.tile([128, DB, 128], BF16, tag="w")
                            nc.sync.dma_start(out=wu_t, in_=ap["wu"][fb]
                                              .rearrange("p (db m) -> p db m", m=128))
                        gp = mlpps.tile([128, 512], F32, tag="gp")
                        up = mlpps.tile([128, 512], F32, tag="up")
                        for db in range(DB):
                            nc.tensor.matmul(gp[:], wg_t[:, db, :],
                                             x1T[:, db, :],
                                             start=(db == 0), stop=(db == DB - 1))
                        for db in range(DB):
                            nc.tensor.matmul(up[:], wu_t[:, db, :],
                                             x1T[:, db, :],
                                             start=(db == 0), stop=(db == DB - 1))
                        gs = mlps.tile([128, 512], F32, tag="gs")
                        nc.vector.tensor_tensor(out=gs[:], in0=gp[:],
                                                in1=rstd2_bc[:], op=AL.mult)
                        us = mlps.tile([128, 512], F32, tag="us")
                        nc.vector.tensor_tensor(out=us[:], in0=up[:],
                                                in1=rstd2_bc[:], op=AL.mult)
                        sg = mlps.tile([128, 512], F32, tag="sg")
                        nc.scalar.activation(out=sg[:], in_=gs[:], func=AF.Silu)
                        nc.vector.tensor_tensor(out=had[:, fi, :], in0=us[:],
                                                in1=sg[:], op=AL.mult)
                    for db in range(DB):
                        wd_t = mlpw.tile([128, FFQ, 128], BF16, tag="w")
                        nc.sync.dma_start(
                            out=wd_t,
                            in_=ap["wd"][qt, db]
                            .rearrange("p (fb m) -> p fb m", m=128))
                        dp = mlpps.tile([128, 512], F32, tag="gp")
                        for fi in range(FFQ):
                            nc.tensor.matmul(dp[:], wd_t[:, fi, :], had[:, fi, :],
                                             start=(fi == 0), stop=(fi == FFQ - 1),
                                             skip_group_check=True)
                        if qt == 0:
                            nc.scalar.copy(x2T[:, db, :], dp[:])
                        elif qt < 3:
                            nc.vector.tensor_tensor(out=x2T[:, db, :],
                                                    in0=x2T[:, db, :].bitcast(F32),
                                                    in1=dp[:], op=AL.add)
                        else:
                            # last quarter: finish x2T, add the residual, and
                            # stream the transposed output out as we go
                            nc.vector.tensor_tensor(out=x2T[:, db, :],
                                                    in0=x2T[:, db, :].bitcast(F32),
                                                    in1=dp[:], op=AL.add)
                            nc.vector.tensor_tensor(out=x2T[:, db, :],
                                                    in0=x2T[:, db, :].bitcast(F32),
                                                    in1=x1T[:, db, :], op=AL.add)
                            if db % 4 == 3:
                                g = db // 4
                                for r in range(4):
                                    tp = st5ps.tile([128, 4, 128], F32R, tag="tp5")
                                    for j in range(4):
                                        db2 = g * 4 + j
                                        nc.tensor.matmul(
                                            tp[:, j, :],
                                            x2T[:, db2, r * 128:(r + 1) * 128],
                                            ident_r[:], is_transpose=True,
                                            start=True, stop=True,
                                            skip_group_check=True)
                                    onat = st5.tile([128, 512], F32, tag="onat")
                                    if r % 2 == 0:
                                        nc.scalar.copy(onat[:],
                                                       tp[:].rearrange("p a c -> p (a c)").bitcast(F32))
                                    else:
                                        nc.vector.tensor_copy(
                                            onat[:], tp[:].rearrange("p a c -> p (a c)").bitcast(F32))
                                    (nc.scalar if r % 2 == 0 else nc.sync).dma_start(
                                        out=ap["out"][r * 128:(r + 1) * 128,
                                                      g * 512:(g + 1) * 512],
                                        in_=onat[:])


def _build(half):
    nc = bacc.Bacc("TRN2", target_bir_lowering=False, debug=False, num_devices=8)
    ap = {}

    def din(name, shape, dt=BF16):
        ap[name] = nc.dram_tensor(name, shape, dt, kind="ExternalInput").ap()

    din("xb", [S, D], F32)
    din("cosT", [HD, S], F32)
    din("sinT", [HD, S], F32)
    din("maskT", [2, 2, 128, CH], F32)
    din("identb", [128, 128], BF16)
    din("identr", [128, 128], F32R)
    din("onesr", [1, 128], F32R)
    din("rotT", [128, 128], F32R)
    din("wq", [H, 128, D]); din("wk", [H, 128, D]); din("wv", [H, 128, D])
    din("wo", [DB, 128, D])
    din("wg", [FFB, 128, D]); din("wu", [FFB, 128, D])
    din("wd", [4, DB, 128, FFQ * 128])
    ap["out"] = nc.dram_tensor("out", [512, D], F32, kind="ExternalOutput").ap()

    with tile.TileContext(nc) as tc:
        _emit(nc, tc, ap, half)
    nc.compile()
    return nc


def _prep(inputs):
    import ml_dtypes
    inp = {k: np.asarray(v) for k, v in inputs.items()}
    w1 = inp["norm_weight_1"].astype(np.float32)
    w2 = inp["norm_weight_2"].astype(np.float32)
    isq = np.float32(1.0 / np.sqrt(HD))

    def fold(wn, scale=None, nw=None):
        w = inp[f"w_{wn}"].astype(np.float64) + (
            inp[f"w_{wn}_lora_a"].astype(np.float64)
            @ inp[f"w_{wn}_lora_b"].astype(np.float64))
        if nw is not None:
            w = nw[:, None].astype(np.float64) * w
        if scale is not None:
            w = w * scale
        return w.astype(np.float32)

    ident = np.eye(128, dtype=np.float32)
    Rm = np.zeros((128, 128), np.float32)
    for i in range(64):
        Rm[i, i + 64] = -1.0
        Rm[i + 64, i] = 1.0

    def _colmajor(w, nblk):
        # [K, N] -> [N/128, 128(K-part), K/128 * 128] contiguous, bf16
        K, N = w.shape
        return np.ascontiguousarray(
            w.reshape(K // 128, 128, nblk, N // nblk)
            .transpose(2, 1, 0, 3).reshape(nblk, 128, K // 128 * (N // nblk))
            .astype(ml_dtypes.bfloat16))

    wd_f = fold("down")
    wd_r = np.ascontiguousarray(
        wd_f.reshape(4, FFQ, 128, DB, 128).transpose(0, 3, 2, 1, 4)
        .reshape(4, DB, 128, FFQ * 128).astype(ml_dtypes.bfloat16))

    shared = dict(
        wq=_colmajor(fold("q", scale=isq, nw=w1), H),
        wk=_colmajor(fold("k", nw=w1), H),
        wv=_colmajor(fold("v", nw=w1), H),
        wo=_colmajor(fold("o"), DB),
        wg=_colmajor(fold("gate", nw=w2), FFB),
        wu=_colmajor(fold("up", nw=w2), FFB),
        wd=wd_r,
        identb=ident.astype(ml_dtypes.bfloat16),
        identr=ident,
        onesr=np.ones((1, 128), np.float32),
        rotT=np.ascontiguousarray(Rm.T))

    pos = inp["position_ids"].astype(np.int64)
    cos_p = inp["cos"].astype(np.float32)[pos]
    sin_p = inp["sin"].astype(np.float32)[pos]
    mask = inp["attention_mask"].astype(np.float32)[0, 0]
    x = inp["x"].astype(np.float32)

    in_maps = []
    for c in range(8):
        b = c // 2
        half = c % 2
        mT = np.zeros((2, 2, 128, CH), np.float32)
        for ci, qc in enumerate(CHUNKS[half]):
            for j in range(2):
                kb = 2 * qc + j
                mT[ci, j] = mask[qc * CH:(qc + 1) * CH, kb * 128:(kb + 1) * 128].T
        m = dict(shared)
        m.update(xb=np.ascontiguousarray(x[b]),
                 cosT=np.ascontiguousarray(cos_p[b].T),
                 sinT=np.ascontiguousarray(sin_p[b].T),
                 maskT=mT)
        in_maps.append(m)
    return in_maps


def kernel(**inputs):
    in_maps = _prep(inputs)
    if "nc" not in _CACHE:
        _CACHE["nc"] = (_build(0), _build(1))
    nc0, nc1 = _CACHE["nc"]

    res0 = run_bass_kernel_spmd(nc0, [in_maps[c] for c in (0, 2, 4, 6)],
                                core_ids=[0, 2, 4, 6])
    res1 = run_bass_kernel_spmd(nc1, [in_maps[c] for c in (1, 3, 5, 7)],
                                core_ids=[1, 3, 5, 7])

    out = np.zeros((B, S, D), np.float32)
    for res, half, cores in ((res0, 0, (0, 2, 4, 6)), (res1, 1, (1, 3, 5, 7))):
        for gi, c in enumerate(cores):
            b = c // 2
            r = res.results[gi]["out"]
            for ci, qc in enumerate(CHUNKS[half]):
                out[b, qc * CH:(qc + 1) * CH] = r[ci * CH:(ci + 1) * CH]
    return out
